# revision 1
# baseline (speedup 1.0000x reference)
"""Causal self-attention Trainium2 kernel (B=4, T=2048, C=1024, H=16, D=64).

Sharding: 8 cores = 4 batches x 2 head-groups (8 heads each).
Each core computes qkv for its head group (column-split w_attn), full causal
attention for its 8 heads, and a partial c_proj (row-split w_proj).  The
partial outputs of the two head-group cores of each batch are summed on the
host (cheaper than a 2-rank on-device all-reduce), and b_proj is added there.

Device layout per core:
  x''      [2048, 1152]  = [x_b | ones | 0-pad]       (bias via matmul)
  w_qkv    [1152, 1536]  = [[w_attn cols for group]; [b_attn]; [0-pad]]
  w_o      [512, 1024]   = w_proj rows for group
  mask     [128, 128]    = upper-triangular (c >= r) multiplicative mask

Pipeline: PE-transpose x -> x^T ; qkv^T matmuls (f32r) giving q^T,k^T in
[D,T] layout and v in [T,D] layout (+ ones column for the softmax
denominator); S^T = (K^T)^T Q^T per 128-k x 512-q block with heads packed
2-per-PE (row groups 0:64 / 64:128); exp on ACT (scale=1/8) -> P^T with the
diagonal 128x128 block masked multiplicatively on DVE afterwards (columns
left of the diagonal are simply excluded from the restricted matmuls);
O^T = [V|1]^T P^T accumulated over k-tiles, row 64 is the softmax
denominator; normalize via DVE reciprocal + gpsimd partition broadcast,
writing y^T over the dead q^T storage; proj matmuls straight from that
y^T layout.
"""

import numpy as np

import concourse.bass as bass
import concourse.tile as tile
import concourse.mybir as mybir
from concourse import bacc, bass_utils
from concourse.masks import make_identity

F32 = mybir.dt.float32
F32R = mybir.dt.float32r
AF = mybir.ActivationFunctionType

B, T, C = 4, 2048, 1024
H = 16            # total heads
HG = 8            # heads per core (head group)
D = 64
CIN = 1152        # padded contraction: 1024 + 1 bias row + padding
NCI = CIN // 128  # 9 contraction tiles
NTT = T // 128    # 16 t tiles
NTB = 4           # t blocks of 512
NQ = 4            # q blocks of 512
NKJ = 16          # k tiles of 128

_NC_CACHE = {}


def build_kernel():
    nc = bacc.Bacc("TRN2", target_bir_lowering=False, debug=False)
    x_d = nc.dram_tensor("x", [T, CIN], F32, kind="ExternalInput").ap()
    wqkv_d = nc.dram_tensor("wqkv", [CIN, 1536], F32, kind="ExternalInput").ap()
    wo_d = nc.dram_tensor("wo", [512, C], F32, kind="ExternalInput").ap()
    mask_d = nc.dram_tensor("mask", [128, 128], F32, kind="ExternalInput").ap()
    out_d = nc.dram_tensor("out", [T, C], F32, kind="ExternalOutput").ap()

    with tile.TileContext(nc) as tc:
        with tc.tile_pool(name="persist", bufs=1) as persist:
            ident = persist.tile([128, 128], F32)
            make_identity(nc, ident[:])
            ones_f = persist.tile([128, 1], F32)
            nc.vector.memset(ones_f[:], 1.0)
            mask_sb = persist.tile([128, 128], F32R)
            nc.gpsimd.dma_start(mask_sb[:], mask_d)

            # q^T / k^T, head-pair-stacked: index r: 0-3 = q pairs, 4-7 = k
            # pairs.  After attention for pair hp, y^T overwrites the q half.
            qkT = persist.tile([128, 8, T], F32R)
            # v + ones column, per k-tile: [tt][h*65:(h+1)*65] = [V_h | 1]
            v_all = persist.tile([128, NKJ, 520], F32R)

            # ---------------- phase 1+2: transpose x and compute qkv ------
            with (
                tc.tile_pool(name="qkv_sb", bufs=2) as qkv_sb,
                tc.tile_pool(name="xnat", bufs=4) as xnat_pool,
                tc.tile_pool(name="wqk", bufs=2) as wqk_pool,
                tc.tile_pool(name="pst", bufs=2, space="PSUM") as pst_pool,
                tc.tile_pool(name="psv", bufs=2, space="PSUM") as psv_pool,
                tc.tile_pool(name="psq", bufs=3, space="PSUM") as psq_pool,
            ):
                # w_v stays resident: [128, 9, 512] f32r (cast during DMA)
                wv = persist.tile([128, NCI, 512], F32R)
                nc.gpsimd.dma_start(
                    wv[:],
                    wqkv_d[:, 1024:1536].rearrange("(ci p) co -> p ci co", p=128),
                )

                for tb in range(NTB):
                    xT = qkv_sb.tile([128, NCI, 512], F32R, tag="xT")
                    for tl in range(4):
                        tt = tb * 4 + tl
                        xn = xnat_pool.tile([128, CIN], F32)
                        nc.sync.dma_start(xn[:], x_d[tt * 128:(tt + 1) * 128, :])
                        for g0, glen in ((0, 4), (4, 4), (8, 1)):
                            pst = pst_pool.tile([128, 512], F32)
                            for cj in range(glen):
                                ci = g0 + cj
                                nc.tensor.transpose(
                                    pst[:, cj * 128:(cj + 1) * 128],
                                    xn[:, ci * 128:(ci + 1) * 128],
                                    ident[:],
                                )
                            nc.scalar.copy(
                                xT[:, g0:g0 + glen, tl * 128:(tl + 1) * 128],
                                pst[:, 0:glen * 128].rearrange(
                                    "p (g c) -> p g c", c=128),
                            )
                    # v for the 4 t-tiles of this block
                    for tl in range(4):
                        tt = tb * 4 + tl
                        psv = psv_pool.tile([128, 512], F32)
                        for ci in range(NCI):
                            nc.tensor.matmul(
                                psv[:], xT[:, ci, tl * 128:(tl + 1) * 128],
                                wv[:, ci, :],
                                start=(ci == 0), stop=(ci == NCI - 1),
                            )
                        vrow = v_all[:, tt, :].rearrange("p (h x) -> p h x", x=65)
                        nc.scalar.copy(
                            vrow[:, :, 0:64],
                            psv[:].rearrange("p (h d) -> p h d", d=64),
                        )
                        nc.vector.tensor_copy(
                            vrow[:, :, 64], ones_f[:].to_broadcast([128, 8]),
                        )
                    # q^T / k^T rows for this t block (two 512-col passes)
                    for half in range(2):
                        wqk = wqk_pool.tile([128, NCI, 512], F32R, tag="wqk")
                        nc.gpsimd.dma_start(
                            wqk[:],
                            wqkv_d[:, half * 512:(half + 1) * 512].rearrange(
                                "(ci p) co -> p ci co", p=128),
                        )
                        for rl in range(4):
                            r = half * 4 + rl
                            psq = psq_pool.tile([128, 512], F32, tag="psq")
                            for ci in range(NCI):
                                nc.tensor.matmul(
                                    psq[:], wqk[:, ci, rl * 128:(rl + 1) * 128],
                                    xT[:, ci, :],
                                    start=(ci == 0), stop=(ci == NCI - 1),
                                )
                            nc.scalar.copy(
                                qkT[:, r, tb * 512:(tb + 1) * 512], psq[:])

            # ---------------- phase 3: attention + projection -------------
            with (
                tc.tile_pool(name="attn_sb", bufs=7) as attn_sb,
                tc.tile_pool(name="norm_sb", bufs=8) as norm_sb,
                tc.tile_pool(name="wo_sb", bufs=1) as wo_pool,
                tc.tile_pool(name="pss", bufs=3, space="PSUM") as pss_pool,
                tc.tile_pool(name="pso", bufs=1, space="PSUM") as pso_pool,
                tc.tile_pool(name="io", bufs=4) as io_pool,
            ):
                wo_t = wo_pool.tile([128, 4, C], F32R)
                nc.gpsimd.dma_start(
                    wo_t[:], wo_d.rearrange("(cc p) co -> p cc co", p=128))

                for hp in range(4):
                    for i in range(NQ):
                        oa = pso_pool.tile([65, 512], F32, tag="oa")
                        ob = pso_pool.tile([65, 512], F32, tag="ob")
                        njt = 4 * i + 4
                        for j in range(njt):
                            m = j - 4 * i          # >=0 on diagonal k-tiles
                            off = max(m, 0) * 128  # first useful q column
                            # head A's block sits at psum cols [off, 512);
                            # head B's is shifted left to [512, 1024-off) so
                            # the written region is contiguous and one exp
                            # covers both heads.
                            ps = pss_pool.tile([128, 1024], F32, tag="ps")
                            nc.tensor.matmul(
                                ps[:, off:512],
                                qkT[0:64, 4 + hp, j * 128:(j + 1) * 128],
                                qkT[0:64, hp, i * 512 + off:(i + 1) * 512],
                                start=True, stop=True,
                            )
                            nc.tensor.matmul(
                                ps[:, 512:1024 - off],
                                qkT[64:128, 4 + hp, j * 128:(j + 1) * 128],
                                qkT[64:128, hp, i * 512 + off:(i + 1) * 512],
                                start=True, stop=True,
                            )
                            ee = attn_sb.tile([128, 1024], F32R, tag="E")
                            nc.scalar.activation(
                                ee[:, off:1024 - off], ps[:, off:1024 - off],
                                AF.Exp, scale=0.125)
                            if m >= 0:
                                nc.vector.tensor_mul(
                                    ee[:, off:off + 128],
                                    ee[:, off:off + 128], mask_sb[:])
                                nc.vector.tensor_mul(
                                    ee[:, 512:640],
                                    ee[:, 512:640], mask_sb[:])
                            vrow = v_all[:, j, :].rearrange(
                                "p (h x) -> p h x", x=65)
                            nc.tensor.matmul(
                                oa[:, off:512],
                                vrow[:, 2 * hp, :],
                                ee[:, off:512],
                                start=(j == 0), stop=(j == njt - 1),
                                skip_group_check=True,
                            )
                            nc.tensor.matmul(
                                ob[:, off:512],
                                vrow[:, 2 * hp + 1, :],
                                ee[:, 512:1024 - off],
                                start=(j == 0), stop=(j == njt - 1),
                                skip_group_check=True,
                            )
                        # normalize; y^T overwrites the dead q^T columns
                        for hh, po in ((0, oa), (1, ob)):
                            rc = norm_sb.tile([1, 512], F32, tag="recip")
                            nc.vector.reciprocal(rc[:], po[64:65, :])
                            bc = norm_sb.tile([64, 512], F32, tag="bcast")
                            nc.gpsimd.partition_broadcast(bc[:], rc[:])
                            nc.vector.tensor_mul(
                                qkT[64 * hh:64 * hh + 64,
                                    hp, i * 512:(i + 1) * 512],
                                po[0:64, :], bc[:])

                # projection: out[t, :] = sum_cc yT[:, cc, t]^T @ wo
                for tt in range(NTT):
                    po = pss_pool.tile([128, 1024], F32, tag="ps")
                    for cc in range(4):
                        for nb in range(2):
                            nc.tensor.matmul(
                                po[:, nb * 512:(nb + 1) * 512],
                                qkT[:, cc, tt * 128:(tt + 1) * 128],
                                wo_t[:, cc, nb * 512:(nb + 1) * 512],
                                start=(cc == 0), stop=(cc == 3),
                                skip_group_check=True,
                            )
                    osb = io_pool.tile([128, 1024], F32, tag="osb")
                    if tt % 2 == 0:
                        nc.vector.tensor_copy(osb[:], po[:])
                    else:
                        nc.scalar.copy(osb[:], po[:])
                    nc.sync.dma_start(out_d[tt * 128:(tt + 1) * 128, :], osb[:])

    nc.compile()
    return nc


def _get_nc():
    if "nc" not in _NC_CACHE:
        _NC_CACHE["nc"] = build_kernel()
    return _NC_CACHE["nc"]


def _make_mask() -> np.ndarray:
    p = np.arange(128)[:, None]
    c = np.arange(128)[None, :]
    return (c >= p).astype(np.float32)


def _make_in_maps(inputs):
    """Build the 8 per-core input dicts from the full (unsharded) inputs."""
    x = np.asarray(inputs["x"], dtype=np.float32)
    w_attn = np.asarray(inputs["w_attn"], dtype=np.float32)
    b_attn = np.asarray(inputs["b_attn"], dtype=np.float32)
    w_proj = np.asarray(inputs["w_proj"], dtype=np.float32)
    mask = _make_mask()
    in_maps = []
    for core in range(8):
        b, g = core // 2, core % 2
        xp = np.zeros((T, CIN), dtype=np.float32)
        xp[:, 0:C] = x[b]
        xp[:, C] = 1.0
        cols = np.concatenate([
            np.arange(g * 512, (g + 1) * 512),
            np.arange(C + g * 512, C + (g + 1) * 512),
            np.arange(2 * C + g * 512, 2 * C + (g + 1) * 512),
        ])
        wq = np.zeros((CIN, 1536), dtype=np.float32)
        wq[0:C, :] = w_attn[:, cols]
        wq[C, :] = b_attn[cols]
        wo = np.ascontiguousarray(w_proj[g * 512:(g + 1) * 512, :])
        in_maps.append({"x": xp, "wqkv": wq, "wo": wo, "mask": mask})
    return in_maps


def kernel(x, w_attn, b_attn, w_proj, b_proj):
    b_proj = np.asarray(b_proj, dtype=np.float32)
    in_maps = _make_in_maps({"x": x, "w_attn": w_attn, "b_attn": b_attn,
                             "w_proj": w_proj})
    nc = _get_nc()
    res = bass_utils.run_bass_kernel_spmd(nc, in_maps, core_ids=list(range(8)))
    out = np.empty((B, T, C), dtype=np.float32)
    for b in range(B):
        out[b] = res.results[2 * b]["out"] + res.results[2 * b + 1]["out"] + b_proj
    return out



# revision 4
# speedup vs baseline: 1.0936x; 1.0936x over previous
"""Causal self-attention Trainium2 kernel (B=4, T=2048, C=1024, H=16, D=64).

Sharding: 8 cores = 4 batches x 2 head-groups (8 heads each).  Each core
computes qkv for its head group, full causal attention for its 8 heads, and a
partial c_proj (row-split w_proj); the two partials per batch are summed on
the host together with b_proj and the (exact) v-bias correction
b_v @ w_proj[group rows].

All data movement that a previous revision did on-device (x transpose, weight
column permutations) is done on the HOST: every device input tensor is
pre-packed into its final SBUF layout so each DMA is a contiguous
[128, free] copy.

Precision scheme (validated vs the f32 reference, rel-l2 ~1.0e-2):
  x^T, w          bf16
  q^T, k^T        fp8e4 (e4m3), packed for DoubleRow matmuls
  S = q^T k       fp8 DoubleRow (2 contraction tiles of d=32 per instr)
  P = exp(S/8)    bf16 (ACT), causal mask via multiplicative bf16 mask (DVE)
  O = [V|1]^T P   bf16 matmul, f32 psum; row 64 = softmax denominator
  y = O * (1/d)   DVE mul with Pool-broadcast reciprocal, bf16 out
  out = y^T @ wo  bf16 matmul, bf16 out, f32 host gather

Device layouts (per core, partition dim first):
  xt_sb    [128, 8ci, 512]        bf16  x^T tile per 512-token block
  wqk_sb   [128, 8ci, 2qk, 2qd, 2half, 128]  bf16 (psum-row-permuted w cols)
  wv_sb    [128, 8ci, 512]        bf16
  wo_sb    [128, 4cc, 1024]       bf16
  qk_sb    [128, 2qk, 2qd, 2half, 2048] fp8e4: head h -> partition band
           32*(h%4), quad h//4; d = (p%32) + 32*half
  v_sb     [128, 16tt, 8h, 65]    bf16 ([V | 1] per k-tile)
  y_sb     [128, 4cc, 2048]       bf16 (c = cc*128 + p; head = 2cc + p//64)

The q^T/k^T psum tiles come out of the QKV matmul with w columns permuted so
that psum partition p = head band (p//32) with d = (p%32) (+32 for the "hi"
half written to psum cols 512:1024); one ACT Identity(+per-partition b_attn
bias) copy per half casts straight into the fp8 DoubleRow layout.

Emission is software-pipelined on the 512-token block axis i:
QKV(0), QKV(1), then ATTN(i) head-pair chunks interleaved with QKV(i+2)
pieces and PROJ(i-1) pieces so the PE stream always has ready work while the
ACT engine drains the exp volume.
"""

import numpy as np
import ml_dtypes

import concourse.bass as bass
import concourse.tile as tile
import concourse.mybir as mybir
from concourse import bacc, bass_utils

F32 = mybir.dt.float32
BF16 = mybir.dt.bfloat16
FP8 = mybir.dt.float8e4
AF = mybir.ActivationFunctionType
DR = mybir.MatmulPerfMode.DoubleRow

B, T, C = 4, 2048, 1024
H = 16            # total heads
D = 64
NCI = 8           # contraction tiles (c = 1024)
NTT = T // 128    # 16 k-tiles
NB = 4            # 512-token blocks

_NC_CACHE = {}

BF = ml_dtypes.bfloat16
F8 = ml_dtypes.float8_e4m3


def build_kernel():
    nc = bacc.Bacc("TRN2", target_bir_lowering=False, debug=False)
    xt_d = nc.dram_tensor("xt", [NB, 128, NCI, 512], BF16, kind="ExternalInput").ap()
    wqk_d = nc.dram_tensor("wqk", [128, NCI, 2, 2, 2, 128], BF16, kind="ExternalInput").ap()
    wv_d = nc.dram_tensor("wv", [128, NCI, 512], BF16, kind="ExternalInput").ap()
    wo_d = nc.dram_tensor("wo", [128, 4, 1024], BF16, kind="ExternalInput").ap()
    bqk_d = nc.dram_tensor("bqk", [128, 2, 2, 2], F32, kind="ExternalInput").ap()
    mask_d = nc.dram_tensor("mask", [128, 128], BF16, kind="ExternalInput").ap()
    out_d = nc.dram_tensor("out", [T, C], BF16, kind="ExternalOutput").ap()

    with tile.TileContext(nc) as tc:
        with (
            tc.tile_pool(name="persist", bufs=1) as persist,
            tc.tile_pool(name="xt", bufs=2) as xt_pool,
            tc.tile_pool(name="ee", bufs=6) as ee_pool,
            tc.tile_pool(name="nrm", bufs=4) as nrm_pool,
            tc.tile_pool(name="osb", bufs=3) as osb_pool,
            tc.tile_pool(name="ps", bufs=2, space="PSUM") as ps_pool,
            tc.tile_pool(name="psv", bufs=2, space="PSUM") as psv_pool,
            tc.tile_pool(name="pso", bufs=1, space="PSUM") as pso_pool,
        ):
            mask_sb = persist.tile([128, 128], BF16)
            nc.sync.dma_start(mask_sb[:], mask_d)
            bqk_sb = persist.tile([128, 2, 2, 2], F32)
            nc.sync.dma_start(bqk_sb[:], bqk_d)
            wqk_sb = persist.tile([128, NCI, 2, 2, 2, 128], BF16)
            nc.sync.dma_start(wqk_sb[:, 0:4], wqk_d[:, 0:4])
            nc.sync.dma_start(wqk_sb[:, 4:8], wqk_d[:, 4:8])
            wv_sb = persist.tile([128, NCI, 512], BF16)
            nc.sync.dma_start(wv_sb[:], wv_d)
            wo_sb = persist.tile([128, 4, 1024], BF16)
            nc.sync.dma_start(wo_sb[:], wo_d)

            qk_sb = persist.tile([128, 2, 2, 2, T], FP8)
            v_sb = persist.tile([128, NTT, 8, 65], BF16)
            nc.vector.memset(v_sb[:, :, :, 64], 1.0)
            y_sb = persist.tile([128, 4, T], BF16)

            xt_tiles = {}

            def load_xt(i):
                xt = xt_pool.tile([128, NCI, 512], BF16, tag="xt")
                nc.sync.dma_start(xt[:, 0:4], xt_d[i, :, 0:4])
                nc.sync.dma_start(xt[:, 4:8], xt_d[i, :, 4:8])
                xt_tiles[i] = xt

            def qkv_qk(i, qd):
                """q^T and k^T for head-quad qd, token block i."""
                xt = xt_tiles[i]
                for qk in range(2):
                    ps = ps_pool.tile([128, 1024], F32, tag="big")
                    for half in range(2):
                        for ci in range(NCI):
                            nc.tensor.matmul(
                                ps[:, half * 512:(half + 1) * 512],
                                wqk_sb[:, ci, qk, qd, half, :],
                                xt[:, ci, :],
                                start=(ci == 0), stop=(ci == NCI - 1),
                            )
                    for half in range(2):
                        nc.scalar.activation(
                            qk_sb[:, qk, qd, half, i * 512:(i + 1) * 512],
                            ps[:, half * 512:(half + 1) * 512],
                            AF.Identity,
                            bias=bqk_sb[:, qk, qd, half:half + 1],
                        )

            def qkv_v(i, tl):
                """v rows for k-tile tt = 4i + tl."""
                xt = xt_tiles[i]
                tt = 4 * i + tl
                psv = psv_pool.tile([128, 512], F32, tag="v")
                for ci in range(NCI):
                    nc.tensor.matmul(
                        psv[:], xt[:, ci, tl * 128:(tl + 1) * 128],
                        wv_sb[:, ci, :],
                        start=(ci == 0), stop=(ci == NCI - 1),
                    )
                nc.vector.tensor_copy(
                    v_sb[:, tt, :, 0:64],
                    psv[:].rearrange("p (h d) -> p h d", d=64),
                )

            def attn(hp, i):
                """Head pair hp (heads 2hp, 2hp+1), query block i."""
                qd, b0 = hp // 2, (2 * hp) % 4
                bA, bB = 32 * b0, 32 * (b0 + 1)
                oa = pso_pool.tile([65, 512], F32, tag="oa")
                ob = pso_pool.tile([65, 512], F32, tag="ob")
                njt = 4 * i + 4
                pending = None

                def emit_o(p):
                    j, ee, off = p
                    vrow = v_sb[:, j, :, :]
                    nc.tensor.matmul(
                        oa[:, off:512], vrow[:, 2 * hp % 8, :], ee[:, off:512],
                        start=(j == 0), stop=(j == njt - 1),
                        skip_group_check=True,
                    )
                    nc.tensor.matmul(
                        ob[:, off:512], vrow[:, (2 * hp + 1) % 8, :],
                        ee[:, 512:1024 - off],
                        start=(j == 0), stop=(j == njt - 1),
                        skip_group_check=True,
                    )

                for j in range(njt):
                    m = j - 4 * i
                    off = max(m, 0) * 128
                    ps = ps_pool.tile([128, 1024], F32, tag="big")
                    nc.tensor.matmul(
                        ps[:, off:512],
                        qk_sb[bA:bA + 32, 1, qd, :, j * 128:(j + 1) * 128],
                        qk_sb[bA:bA + 32, 0, qd, :, i * 512 + off:(i + 1) * 512],
                        start=True, stop=True, perf_mode=DR,
                        tile_position=(bA, 0),
                    )
                    nc.tensor.matmul(
                        ps[:, 512:1024 - off],
                        qk_sb[bB:bB + 32, 1, qd, :, j * 128:(j + 1) * 128],
                        qk_sb[bB:bB + 32, 0, qd, :, i * 512 + off:(i + 1) * 512],
                        start=True, stop=True, perf_mode=DR,
                        tile_position=(bB, 0),
                    )
                    ee = ee_pool.tile([128, 1024], BF16, tag="E")
                    nc.scalar.activation(
                        ee[:, off:1024 - off], ps[:, off:1024 - off],
                        AF.Exp, scale=0.125)
                    if m >= 0:
                        nc.vector.tensor_mul(
                            ee[:, off:off + 128], ee[:, off:off + 128],
                            mask_sb[:])
                        nc.vector.tensor_mul(
                            ee[:, 512:640], ee[:, 512:640], mask_sb[:])
                    if pending is not None:
                        emit_o(pending)
                    pending = (j, ee, off)
                emit_o(pending)

                # normalize: y = O * (1/denom); y_sb rows 0:64 head A,
                # 64:128 head B of chunk cc = hp
                for hh, po in ((0, oa), (1, ob)):
                    rc = nrm_pool.tile([1, 512], F32, tag="rc")
                    nc.vector.reciprocal(rc[:], po[64:65, :])
                    bc = nrm_pool.tile([64, 512], F32, tag="bc")
                    nc.gpsimd.partition_broadcast(bc[:], rc[:])
                    nc.vector.tensor_mul(
                        y_sb[64 * hh:64 * hh + 64, hp, i * 512:(i + 1) * 512],
                        po[0:64, :], bc[:])

            def proj(tt):
                po = ps_pool.tile([128, 1024], F32, tag="big")
                for cc in range(4):
                    for nb in range(2):
                        nc.tensor.matmul(
                            po[:, nb * 512:(nb + 1) * 512],
                            y_sb[:, cc, tt * 128:(tt + 1) * 128],
                            wo_sb[:, cc, nb * 512:(nb + 1) * 512],
                            start=(cc == 0), stop=(cc == 3),
                            skip_group_check=True,
                        )
                osb = osb_pool.tile([128, 1024], BF16, tag="o")
                nc.vector.tensor_copy(osb[:], po[:])
                nc.sync.dma_start(out_d[tt * 128:(tt + 1) * 128, :], osb[:])

            # ---------------- software-pipelined emission ----------------
            load_xt(0)
            load_xt(1)
            for qd in range(2):
                qkv_qk(0, qd)
            for tl in range(4):
                qkv_v(0, tl)

            # steady state: per i, attention chunks interleaved with
            # QKV(i+1) pieces and PROJ(i-1) pieces
            for i in range(NB):
                nxt = i + 1
                if nxt < NB:
                    load_xt(nxt)
                pieces = []
                if nxt < NB:
                    pieces = [
                        lambda qd=0: qkv_qk(nxt, qd),
                        lambda qd=1: qkv_qk(nxt, qd),
                        lambda: [qkv_v(nxt, tl) for tl in (0, 1)],
                        lambda: [qkv_v(nxt, tl) for tl in (2, 3)],
                    ]
                prj = []
                if i > 0:
                    prj = [lambda tt=4 * (i - 1) + t: proj(tt) for t in range(4)]
                for hp in range(4):
                    attn(hp, i)
                    if hp < len(pieces):
                        pieces[hp]()
                    if hp < len(prj):
                        prj[hp]()
            for tt in range(12, 16):
                proj(tt)

    nc.compile()
    return nc


def _get_nc():
    if "nc" not in _NC_CACHE:
        _NC_CACHE["nc"] = build_kernel()
    return _NC_CACHE["nc"]


def _make_mask() -> np.ndarray:
    p = np.arange(128)[:, None]
    c = np.arange(128)[None, :]
    return (c >= p).astype(BF)


def _make_in_maps(inputs):
    """Build the 8 per-core input dicts, pre-packed into SBUF layouts."""
    x = np.asarray(inputs["x"], dtype=np.float32)
    w_attn = np.asarray(inputs["w_attn"], dtype=np.float32)
    b_attn = np.asarray(inputs["b_attn"], dtype=np.float32)
    w_proj = np.asarray(inputs["w_proj"], dtype=np.float32)
    mask = _make_mask()

    # psum-row permutation for q/k: row r (+512*half) -> w_attn column
    # qk*1024 + g*512 + (qd*4 + r//32)*64 + r%32 + 32*half
    r = np.arange(128)
    qk_i = np.arange(2)[:, None, None, None]
    qd_i = np.arange(2)[None, :, None, None]
    hf_i = np.arange(2)[None, None, :, None]
    col0 = qk_i * 1024 + (qd_i * 4 + r[None, None, None, :] // 32) * 64 \
        + r[None, None, None, :] % 32 + 32 * hf_i          # [2,2,2,128], no g

    in_maps = []
    for core in range(8):
        b, g = core // 2, core % 2
        cols = col0 + g * 512                                # [2,2,2,128]

        # x^T packed: [4, 128, 8, 512]; xt[tb, p, ci, tw] = x[b, tb*512+tw, ci*128+p]
        xt = np.ascontiguousarray(
            x[b].reshape(4, 512, 8, 128).transpose(0, 3, 2, 1)).astype(BF)

        # wqk: [128, 8, 2, 2, 2, 128]
        w8 = w_attn.reshape(8, 128, 3 * C)                   # [ci, p, col]
        wqk = np.ascontiguousarray(
            w8[:, :, cols.reshape(-1)].reshape(8, 128, 2, 2, 2, 128)
            .transpose(1, 0, 2, 3, 4, 5)).astype(BF)

        wv = np.ascontiguousarray(
            w8[:, :, 2048 + g * 512: 2048 + (g + 1) * 512]
            .transpose(1, 0, 2)).astype(BF)                  # [128, 8, 512]

        wo = np.ascontiguousarray(
            w_proj[g * 512:(g + 1) * 512, :]
            .reshape(4, 128, 1024).transpose(1, 0, 2)).astype(BF)

        bqk = np.ascontiguousarray(
            b_attn[cols].transpose(3, 0, 1, 2)).astype(np.float32)

        in_maps.append({"xt": xt, "wqk": wqk, "wv": wv, "wo": wo,
                        "bqk": bqk, "mask": mask})
    return in_maps


def kernel(x, w_attn, b_attn, w_proj, b_proj):
    b_attn = np.asarray(b_attn, dtype=np.float32)
    b_proj = np.asarray(b_proj, dtype=np.float32)
    w_proj_f = np.asarray(w_proj, dtype=np.float32)
    in_maps = _make_in_maps({"x": x, "w_attn": w_attn, "b_attn": b_attn,
                             "w_proj": w_proj})
    nc = _get_nc()
    res = bass_utils.run_bass_kernel_spmd(nc, in_maps, core_ids=list(range(8)))

    # exact v-bias correction: y_group = O/d + b_v  =>  out += b_v @ w_o[group]
    bias_term = np.empty((2, C), dtype=np.float32)
    for g in range(2):
        bv = b_attn[2048 + g * 512: 2048 + (g + 1) * 512]
        bias_term[g] = bv @ w_proj_f[g * 512:(g + 1) * 512, :]

    out = np.empty((B, T, C), dtype=np.float32)
    for b in range(B):
        out[b] = (res.results[2 * b]["out"].astype(np.float32)
                  + res.results[2 * b + 1]["out"].astype(np.float32)
                  + bias_term[0] + bias_term[1] + b_proj)
    return out
